# revision 1
# baseline (speedup 1.0000x reference)
"""Trainium2 Bass kernel for a 2-layer xLSTM (sLSTM -> mLSTM).

Strategy (8 NeuronCores, data-parallel over batch, 1 batch element/core):
  - Layer 0 (sLSTM): serial scan over T=1024. State kept UNNORMALIZED
    (c,n scaled by exp(m)) so no per-step max-stabilizer chain is needed:
      cn = exp(ft)*cn + exp(it - mu)*[z, 1]
    Every 16 steps the state is rescaled by an exact power of two
    (exponent-field extraction) and the log-offset mu is folded into the
    future i-gate pre-activations (Gx patch). h = 0.5*(1+tanh(o/2))*c/n
    with the o-gate weights pre-halved (sigmoid via tanh).
  - Layer 1 (mLSTM): chunkwise-parallel formulation (chunk L=128). The
    per-channel gates factor as exp(a_s - u_t) with a = i_logit - cumsum(f),
    u = running max(0, a), so each chunk reduces to a handful of 128x128
    matmuls + cumulative scans. Work for chunk c is interleaved into the
    serial sLSTM steps of chunk c+1 where the engines are otherwise idle.

kernel(**inputs) takes the FULL inputs and returns the FULL (B,T,H) output.
"""

import numpy as np

import concourse.bacc as bacc
import concourse.tile as tile
from concourse import mybir
from concourse.masks import make_identity, make_upper_triangular

AF = mybir.ActivationFunctionType
OP = mybir.AluOpType
FP32 = mybir.dt.float32
U32 = mybir.dt.uint32

B, T, I, H = 8, 1024, 128, 128
L = 128
NCHUNK = T // L
RENORM = 16
LN2 = 0.6931471805599453

TRACE = False
LAST_RESULTS = None
_NC_CACHE = {}


def _emit_slstm_step(nc, st, t, psG):
    """One serial sLSTM step. h1[:, t+1] <- step(h1[:, t])."""
    gps = psG.tile([H, 4], FP32, tag="g", name="gps")
    # gate pre-activations: psum = Gx[:, t, :] + sR_g @ h
    nc.tensor.matmul(gps, st["ident"], st["Gx"][:, t, :], start=True, stop=False)
    for g in range(4):
        nc.tensor.matmul(
            gps[:, g : g + 1],
            st["sRT4"][:, g * H : (g + 1) * H],
            st["h1"][:, t : t + 1],
            start=False,
            stop=(g == 3),
        )
    # eif = exp(psum[:, i,f]); z1[:,0:2] = tanh(psum[:, z,o])
    nc.scalar.activation(st["eif"], gps[:, 0:2], AF.Exp)
    nc.scalar.activation(st["z1"][:, 0:2], gps[:, 2:4], AF.Tanh)
    # iz1 = ei * [z, 1]
    nc.vector.tensor_scalar(
        st["iz1"], st["z1"][:, 0:3:2], st["eif"][:, 0:1], None, OP.mult
    )
    # cn = ef*cn + iz1
    nc.vector.scalar_tensor_tensor(
        st["cn"], st["cn"], st["eif"][:, 1:2], st["iz1"], OP.mult, OP.add
    )
    nc.vector.reciprocal(st["rr"], st["cn"][:, 1:2])
    # cr = 0.5 * c / n
    nc.vector.tensor_scalar(
        st["cr"], st["cn"][:, 0:1], st["rr"], 0.5, OP.mult, OP.mult
    )
    # h = to*cr + cr  (= sigmoid(o)*c/n with o pre-halved)
    nc.vector.scalar_tensor_tensor(
        st["h1"][:, t + 1 : t + 2], st["z1"][:, 1:2], st["cr"], st["cr"],
        OP.mult, OP.add,
    )
    if (t + 1) % RENORM == 0:
        _emit_renorm(nc, st, t)


def _emit_renorm(nc, st, t):
    """Rescale cn by 2^-e2(n) exactly; fold ln of the scale into future Gx_i."""
    cn_u = st["cn"][:, 1:2].bitcast(U32)
    nc.vector.tensor_scalar(
        st["p2"].bitcast(U32), cn_u, 0x7F800000, None, OP.bitwise_and
    )
    nc.vector.tensor_scalar(
        st["e2"].bitcast(U32), cn_u, 23, 0x4B000000,
        OP.logical_shift_right, OP.bitwise_or,
    )
    # negdelta = -(e_biased - 127) * ln2 ; e2 holds 2^23 + e_biased as fp32
    nc.vector.tensor_scalar(
        st["nd"], st["e2"], -8388735.0, -LN2, OP.add, OP.mult
    )
    nc.vector.tensor_tensor(st["negmu"], st["negmu"], st["nd"], OP.add)
    nc.vector.reciprocal(st["rs"], st["p2"])
    nc.vector.tensor_scalar(st["cn"], st["cn"], st["rs"], None, OP.mult)
    if t + 1 < T:
        hi = min(t + 1 + RENORM, T)
        sl = st["Gx"][:, t + 1 : hi, 0:1]
        nc.vector.tensor_scalar(sl, sl, st["negmu"], None, OP.add)


def _mlstm_chunk_ops(nc, st, ci, psB, chk, hout_d):
    """Return a list of closures, each emitting one instruction of mLSTM
    chunk ci. Layouts: channel on partitions ([a, t]) except where noted."""
    s0 = ci * L
    sl = slice(s0, s0 + L)
    h1sl = slice(1 + s0, 1 + s0 + L)
    last = ci == NCHUNK - 1
    ops = []

    # -- projections q,k,v,it,ft,to (o pre-halved, k pre-scaled)
    PROJ = [("q_", AF.Identity), ("k_", AF.Identity), ("v_", AF.Identity),
            ("it_", AF.Identity), ("ft_", AF.Identity), ("tom", AF.Tanh)]

    def mk_proj(j, name, func):
        def mm():
            ps = psB.tile([H, L], FP32, tag="ps", name="proj_ps")
            chk["proj_ps"] = ps
            nc.tensor.matmul(
                ps, st["WT6"][:, j * H : (j + 1) * H], st["h1"][:, h1sl],
                start=True, stop=True,
            )
        def cp():
            nc.scalar.activation(
                st[name][:, sl], chk["proj_ps"], func,
                bias=st["b6"][:, j : j + 1],
            )
        return [mm, cp]

    for j, (name, func) in enumerate(PROJ):
        ops += mk_proj(j, name, func)

    # -- gate scans: F = cumsum(ft); a = it - F; u = runmax(0, a)
    def scan_F():
        init = 0.0 if ci == 0 else st["F_"][:, s0 - 1 : s0]
        nc.vector.tensor_tensor_scan(
            st["F_"][:, sl], st["ft_"][:, sl], st["zerL"], init, OP.add, OP.add
        )
    def calc_a():
        nc.vector.tensor_tensor(
            st["a_"][:, sl], st["it_"][:, sl], st["F_"][:, sl], OP.subtract
        )
    def scan_u():
        init = 0.0 if ci == 0 else st["u_"][:, s0 - 1 : s0]
        nc.vector.tensor_tensor_scan(
            st["u_"][:, sl], st["a_"][:, sl], st["zerL"], init, OP.max, OP.add
        )
    ops += [scan_F, calc_a, scan_u]

    u_end = st["u_"][:, s0 + L - 1 : s0 + L]

    def calc_negu():
        nc.vector.tensor_scalar(st["negu"], u_end, -1.0, None, OP.mult)
    def calc_P():
        nc.scalar.activation(st["Pc"], st["a_"][:, sl], AF.Exp, bias=st["negu"])
    def calc_E():
        nc.scalar.activation(st["Ec"], st["u_"][:, sl], AF.Exp,
                             bias=u_end, scale=-1.0)
    ops += [calc_negu, calc_P, calc_E]

    if ci > 0:
        def calc_d():
            nc.scalar.activation(st["ddec"], st["u_"][:, s0 - 1 : s0], AF.Exp,
                                 bias=st["negu"])
        def scale_Cs():
            nc.vector.tensor_scalar(st["CsS"], st["Cs"], st["ddec"], None, OP.mult)
        def tr_Cs():
            ps = psB.tile([H, H], FP32, tag="ps2", name="cst_ps")
            chk["cst_ps"] = ps
            nc.tensor.transpose(ps, st["CsS"], st["ident"])
        def cp_Cst():
            nc.vector.tensor_copy(st["Cst"], chk["cst_ps"])
        def calc_dn():
            nc.vector.tensor_scalar(
                st["dn"], st["Ncum"][:, s0 - 1 : s0], st["ddec"], None, OP.mult
            )
        ops += [calc_d, scale_Cs, tr_Cs, cp_Cst, calc_dn]

    # -- n accumulation (per-channel cumsum of P*k with decayed carry)
    def calc_PK():
        nc.vector.tensor_tensor(st["PKc"], st["Pc"], st["k_"][:, sl], OP.mult)
    def scan_N():
        init = 0.0 if ci == 0 else st["dn"]
        nc.vector.tensor_tensor_scan(
            st["Ncum"][:, sl], st["PKc"], st["zerL"], init, OP.add, OP.add
        )
    ops += [calc_PK, scan_N]

    # -- attention-style intra-chunk matmuls
    def mm_St():
        ps = psB.tile([L, L], FP32, tag="ps2", name="st_ps")
        chk["st_ps"] = ps
        nc.tensor.matmul(ps, st["k_"][:, sl], st["q_"][:, sl],
                         start=True, stop=True)
    def mask_S():
        nc.vector.tensor_tensor(st["Sm"], chk["st_ps"], st["tri"], OP.mult)
    def calc_PV():
        nc.vector.tensor_tensor(st["PVa"], st["Pc"], st["v_"][:, sl], OP.mult)
    def tr_PV():
        ps = psB.tile([H, L], FP32, tag="ps2", name="t_ps")
        chk["pvt_ps"] = ps
        nc.tensor.transpose(ps, st["PVa"], st["ident"])
    def cp_PVt():
        nc.vector.tensor_copy(st["PVt"], chk["pvt_ps"])
    ops += [mm_St, mask_S, calc_PV, tr_PV, cp_PVt]

    def mm_IH():
        ps = psB.tile([L, H], FP32, tag="ps3", name="ih_ps")
        chk["ih_ps"] = ps
        nc.tensor.matmul(ps, st["Sm"], st["PVt"], start=True, stop=(ci == 0))
    ops.append(mm_IH)
    if ci > 0:
        def mm_carry():
            nc.tensor.matmul(chk["ih_ps"], st["q_"][:, sl], st["Cst"],
                             start=False, stop=True)
        ops.append(mm_carry)

    def cp_IH():
        nc.vector.tensor_copy(st["IHs"], chk["ih_ps"])
    def tr_IH():
        ps = psB.tile([H, L], FP32, tag="ps2", name="t_ps")
        chk["iht_ps"] = ps
        nc.tensor.transpose(ps, st["IHs"], st["ident"])
    ops += [cp_IH, tr_IH]

    # -- denominator: row = sum_a E*Ncum*q ; rec = 0.5/max(|row|, 1)
    def calc_ENQ():
        nc.vector.tensor_tensor(st["ENQ"], st["Ncum"][:, sl], st["q_"][:, sl],
                                OP.mult)
    def calc_ENQ2():
        nc.vector.tensor_tensor(st["ENQ2"], st["ENQ"], st["Ec"], OP.mult)
    def mm_row():
        ps = psB.tile([1, L], FP32, tag="ps4", name="row_ps")
        chk["row_ps"] = ps
        nc.tensor.matmul(ps, st["ones1"], st["ENQ2"], start=True, stop=True)
    def calc_drow():
        nc.scalar.activation(st["drow"], chk["row_ps"], AF.Abs)
    def calc_drow2():
        nc.vector.tensor_scalar(st["drow2"], st["drow"], 1.0, 2.0,
                                OP.max, OP.mult)
    def calc_rrow():
        nc.vector.reciprocal(st["rrow"], st["drow2"])
    def bcast_r():
        nc.gpsimd.partition_broadcast(st["Rb"], st["rrow"])
    ops += [calc_ENQ, calc_ENQ2, mm_row, calc_drow, calc_drow2, calc_rrow,
            bcast_r]

    # -- output: h = (1+to) * E * IH * (0.5/den)
    def calc_EH():
        nc.vector.tensor_tensor(st["EH"], st["Ec"], chk["iht_ps"], OP.mult)
    def calc_EHR():
        nc.vector.tensor_tensor(st["EHR"], st["EH"], st["Rb"], OP.mult)
    def calc_t2():
        nc.vector.tensor_scalar(st["t2"], st["tom"][:, sl], 1.0, None, OP.add)
    def calc_h():
        nc.vector.tensor_tensor(st["houts"][:, sl], st["t2"], st["EHR"], OP.mult)
    def dma_h():
        nc.sync.dma_start(out=hout_d[:, sl], in_=st["houts"][:, sl])
    ops += [calc_EH, calc_EHR, calc_t2, calc_h, dma_h]

    # -- state update for next chunk
    if not last:
        def tr_K():
            ps = psB.tile([H, L], FP32, tag="ps2", name="t_ps")
            chk["kt_ps"] = ps
            nc.tensor.transpose(ps, st["k_"][:, sl], st["ident"])
        def cp_Kt():
            nc.vector.tensor_copy(st["Kts"], chk["kt_ps"])
        def mm_Cd():
            ps = psB.tile([H, H], FP32, tag="ps3", name="cd_ps")
            chk["cd_ps"] = ps
            nc.tensor.matmul(ps, st["PVt"], st["Kts"], start=True, stop=True)
        ops += [tr_K, cp_Kt, mm_Cd]
        if ci == 0:
            def upd_Cs():
                nc.vector.tensor_copy(st["Cs"], chk["cd_ps"])
        else:
            def upd_Cs():
                nc.vector.tensor_tensor(st["Cs"], st["CsS"], chk["cd_ps"], OP.add)
        ops.append(upd_Cs)

    return ops


def _build_body(nc, tc, dram):
    from contextlib import ExitStack

    with ExitStack() as ctx:
        const = ctx.enter_context(tc.tile_pool(name="const", bufs=1))
        psG = ctx.enter_context(tc.tile_pool(name="psG", bufs=2, space="PSUM"))
        psA = ctx.enter_context(tc.tile_pool(name="psA", bufs=2, space="PSUM"))
        psB = ctx.enter_context(tc.tile_pool(name="psB", bufs=1, space="PSUM"))

        st = {}

        def sb(name, shape, dtype=FP32):
            st[name] = const.tile(shape, dtype, tag=name, name=name)
            return st[name]

        # constants / weights
        for name, shape in [
            ("xT", [I, T]), ("sWT4", [I, 4 * H]), ("sRT4", [H, 4 * H]),
            ("sb4", [H, 4]), ("WT6", [H, 6 * H]), ("b6", [H, 6]),
        ]:
            sb(name, shape)
            nc.sync.dma_start(out=st[name], in_=dram[name][:])
        ident = sb("ident", [128, 128]); make_identity(nc, ident[:, :])
        tri = sb("tri", [L, L]); make_upper_triangular(nc, tri[:, :], val=1.0, diag=True)
        sb("zerL", [128, L]); nc.vector.memset(st["zerL"], 0.0)
        sb("ones1", [128, 1]); nc.vector.memset(st["ones1"], 1.0)

        # persistent buffers
        sb("Gx", [H, T, 4])
        sb("h1", [H, T + 1]); nc.vector.memset(st["h1"][:, 0:1], 0.0)
        for name in ["q_", "k_", "v_", "it_", "ft_", "tom", "F_", "a_", "u_",
                     "Ncum", "houts"]:
            sb(name, [H, T])
        for name in ["Cs", "CsS", "Cst"]:
            sb(name, [H, H])
        # sLSTM step state
        sb("cn", [H, 2]); nc.vector.memset(st["cn"], 0.0)
        sb("z1", [H, 3]); nc.vector.memset(st["z1"][:, 2:3], 1.0)
        for name in ["eif", "iz1"]:
            sb(name, [H, 2])
        for name in ["rr", "cr", "p2", "e2", "nd", "negmu", "rs"]:
            sb(name, [H, 1])
        nc.vector.memset(st["negmu"], 0.0)
        # mLSTM chunk scratch
        for name in ["Pc", "Ec", "PKc", "Sm", "PVa", "PVt", "IHs", "Kts",
                     "ENQ", "ENQ2", "Rb", "EH", "EHR", "t2"]:
            sb(name, [128, L])
        for name in ["negu", "ddec", "dn"]:
            sb(name, [H, 1])
        sb("drow", [1, L]); sb("drow2", [1, L]); sb("rrow", [1, L])

        # Gx precompute: Gx[:, tt, g] = sW_g @ x_t (+ sb_g)
        for g in range(4):
            for tt in range(T // 512):
                ps = psA.tile([H, 512], FP32, tag="gx", name="gx_ps")
                nc.tensor.matmul(
                    ps, st["sWT4"][:, g * H : (g + 1) * H],
                    st["xT"][:, tt * 512 : (tt + 1) * 512],
                    start=True, stop=True,
                )
                nc.scalar.activation(
                    st["Gx"][:, tt * 512 : (tt + 1) * 512, g], ps,
                    AF.Identity, bias=st["sb4"][:, g : g + 1],
                )

        # serial loop with interleaved mLSTM chunk work
        chk = {}
        pending = []
        for t in range(T):
            _emit_slstm_step(nc, st, t, psG)
            if pending:
                pending.pop(0)()
            if (t + 1) % L == 0:
                ci = (t + 1) // L - 1
                pending += _mlstm_chunk_ops(nc, st, ci, psB, chk, dram["hout"])
        while pending:
            pending.pop(0)()


def _get_nc():
    if "nc" in _NC_CACHE:
        return _NC_CACHE["nc"]
    nc = bacc.Bacc("TRN2", debug=False, num_devices=B)
    dram = {}
    for name, shape in [
        ("xT", [I, T]), ("sWT4", [I, 4 * H]), ("sRT4", [H, 4 * H]),
        ("sb4", [H, 4]), ("WT6", [H, 6 * H]), ("b6", [H, 6]),
    ]:
        dram[name] = nc.declare_dram_parameter(name, shape, FP32, isOutput=False)
    dram["hout"] = nc.declare_dram_parameter("hout", [H, T], FP32, isOutput=True)
    with tile.TileContext(nc) as tc:
        _build_body(nc, tc, dram)
    nc.compile()
    _NC_CACHE["nc"] = nc
    return nc


def _make_runner(nc):
    """Build a jitted SPMD runner for a compiled Bacc program (replicates
    bass2jax.run_bass_via_pjrt but reuses the jitted callable across calls)."""
    import jax
    from jax.sharding import Mesh, PartitionSpec
    from jax.experimental.shard_map import shard_map
    from concourse import mybir as _mb
    from concourse.bass2jax import (
        _bass_exec_p, install_neuronx_cc_hook, partition_id_tensor,
    )

    install_neuronx_cc_hook()
    partition_name = nc.partition_id_tensor.name if nc.partition_id_tensor else None
    in_names, out_names, out_avals, zero_outs = [], [], [], []
    for alloc in nc.m.functions[0].allocations:
        if not isinstance(alloc, _mb.MemoryLocationSet):
            continue
        name = alloc.memorylocations[0].name
        if alloc.kind == "ExternalInput":
            if name != partition_name:
                in_names.append(name)
        elif alloc.kind == "ExternalOutput":
            out_names.append(name)
            shape = tuple(alloc.tensor_shape)
            dtype = _mb.dt.np(alloc.dtype)
            out_avals.append(jax.core.ShapedArray(shape, dtype))
            zero_outs.append(np.zeros(shape, dtype))
    n_params = len(in_names)
    n_outs = len(out_avals)
    param_names = list(in_names)
    in_names = in_names + out_names
    if partition_name is not None:
        in_names.append(partition_name)

    def _body(*args):
        operands = list(args)
        if partition_name is not None:
            operands.append(partition_id_tensor())
        outs = _bass_exec_p.bind(
            *operands,
            out_avals=tuple(out_avals),
            in_names=tuple(in_names),
            out_names=tuple(out_names),
            lowering_input_output_aliases=(),
            sim_require_finite=True,
            sim_require_nnan=True,
            nc=nc,
        )
        return tuple(outs)

    devices = jax.devices()[:B]
    mesh = Mesh(np.asarray(devices), ("core",))
    in_specs = (PartitionSpec("core"),) * (n_params + n_outs)
    out_specs = (PartitionSpec("core"),) * n_outs
    sharded = jax.jit(
        shard_map(_body, mesh=mesh, in_specs=in_specs, out_specs=out_specs,
                  check_rep=False),
        donate_argnums=tuple(range(n_params, n_params + n_outs)),
        keep_unused=True,
    )

    def run(in_maps):
        concat_in = [
            np.concatenate([np.asarray(m[name]) for m in in_maps], axis=0)
            for name in param_names
        ]
        concat_zeros = [
            np.zeros((B * z.shape[0], *z.shape[1:]), z.dtype) for z in zero_outs
        ]
        out_arrs = sharded(*concat_in, *concat_zeros)
        out_arrs = [np.asarray(a) for a in out_arrs]
        return [
            {name: out_arrs[i].reshape(B, *out_avals[i].shape)[c]
             for i, name in enumerate(out_names)}
            for c in range(B)
        ]

    return run


def _get_runner():
    if "runner" not in _NC_CACHE:
        _NC_CACHE["runner"] = _make_runner(_get_nc())
    return _NC_CACHE["runner"]


def kernel(**inputs):
    global LAST_RESULTS
    f32 = np.float32
    x = np.ascontiguousarray(inputs["x"], dtype=f32)
    sW = np.asarray(inputs["sW"], f32); sR = np.asarray(inputs["sR"], f32)
    sb_ = np.asarray(inputs["sb"], f32)
    inv_sqrt_h = f32(1.0 / np.sqrt(H))

    sWT4 = np.ascontiguousarray(sW.T); sRT4 = np.ascontiguousarray(sR.T)
    sWT4[:, 3 * H :] *= 0.5; sRT4[:, 3 * H :] *= 0.5
    sb4 = np.ascontiguousarray(sb_.reshape(4, H).T)
    sb4[:, 3] *= 0.5

    WT = {}
    bvecs = []
    for j, wn, bn in [(0, "Wq", "bq"), (1, "Wk", "bk"), (2, "Wv", "bv"),
                      (3, "Wi", "bi"), (4, "Wf", "bf"), (5, "Wo", "bo")]:
        w = np.asarray(inputs[wn], f32).T.copy()
        b = np.asarray(inputs[bn], f32).copy()
        if wn == "Wk":
            w *= inv_sqrt_h; b = b * inv_sqrt_h
        if wn == "Wo":
            w *= 0.5; b = b * 0.5
        WT[j] = w
        bvecs.append(b)
    WT6 = np.ascontiguousarray(np.concatenate([WT[j] for j in range(6)], axis=1))
    b6 = np.ascontiguousarray(np.stack(bvecs, axis=1))

    run = _get_runner()
    in_maps = []
    for b_ in range(B):
        in_maps.append({
            "xT": np.ascontiguousarray(x[b_].T),
            "sWT4": sWT4, "sRT4": sRT4, "sb4": sb4, "WT6": WT6, "b6": b6,
        })
    results = run(in_maps)
    LAST_RESULTS = results
    out = np.empty((B, T, H), f32)
    for b_ in range(B):
        out[b_] = results[b_]["hout"].T
    return out



# revision 3
# speedup vs baseline: 26.4272x; 26.4272x over previous
"""Trainium2 Bass kernel for a 2-layer xLSTM (sLSTM -> mLSTM).

Strategy (8 NeuronCores, data-parallel over batch, 1 batch element/core):
  - Layer 0 (sLSTM): serial scan over T=1024. State kept UNNORMALIZED
    (c,n scaled by exp(m)) so no per-step max-stabilizer chain is needed:
      cn = exp(ft)*cn + exp(it - mu)*[z, 1]
    Every 16 steps the state is rescaled by an exact power of two
    (exponent-field extraction) and the log-offset mu is folded into the
    future i-gate pre-activations (Gx patch). h = 0.5*(1+tanh(o/2))*c/n
    with the o-gate weights pre-halved (sigmoid via tanh).
  - Layer 1 (mLSTM): chunkwise-parallel formulation (chunk L=128). The
    per-channel gates factor as exp(a_s - u_t) with a = i_logit - cumsum(f),
    u = running max(0, a), so each chunk reduces to a handful of 128x128
    matmuls + cumulative scans. Work for chunk c is interleaved into the
    serial sLSTM steps of chunk c+1 where the engines are otherwise idle.

kernel(**inputs) takes the FULL inputs and returns the FULL (B,T,H) output.
"""

import numpy as np

import concourse.bacc as bacc
import concourse.tile as tile
from concourse import mybir
from concourse.masks import make_identity, make_upper_triangular

AF = mybir.ActivationFunctionType
OP = mybir.AluOpType
FP32 = mybir.dt.float32
U32 = mybir.dt.uint32

B, T, I, H = 8, 1024, 128, 128
L = 128
NCHUNK = T // L
RENORM = 16
LN2 = 0.6931471805599453

TRACE = False
LAST_RESULTS = None
LAST_IN_MAPS = None
_NC_CACHE = {}


def _emit_slstm_step(nc, st, t, psG):
    """One serial sLSTM step. h1[:, t+1] <- step(h1[:, t])."""
    gps = psG.tile([H, 4], FP32, tag="g", name="gps")
    # gate pre-activations: psum = Gx[:, t, :] + sR_g @ h
    nc.tensor.matmul(gps, st["ident"], st["Gx"][:, t, :], start=True, stop=False)
    for g in range(4):
        nc.tensor.matmul(
            gps[:, g : g + 1],
            st["sRT4"][:, g * H : (g + 1) * H],
            st["h1"][:, t : t + 1],
            start=False,
            stop=(g == 3),
        )
    # eif = exp(psum[:, i,f]); z1[:,0:2] = tanh(psum[:, z,o])
    nc.scalar.activation(st["eif"], gps[:, 0:2], AF.Exp)
    nc.scalar.activation(st["z1"][:, 0:2], gps[:, 2:4], AF.Tanh)
    # iz1 = ei * [z, 1]
    nc.vector.tensor_scalar(
        st["iz1"], st["z1"][:, 0:3:2], st["eif"][:, 0:1], None, OP.mult
    )
    # cn = ef*cn + iz1
    nc.vector.scalar_tensor_tensor(
        st["cn"], st["cn"], st["eif"][:, 1:2], st["iz1"], OP.mult, OP.add
    )
    nc.vector.reciprocal(st["rr"], st["cn"][:, 1:2])
    # cr = 0.5 * c / n
    nc.vector.tensor_scalar(
        st["cr"], st["cn"][:, 0:1], st["rr"], 0.5, OP.mult, OP.mult
    )
    # h = to*cr + cr  (= sigmoid(o)*c/n with o pre-halved)
    nc.vector.scalar_tensor_tensor(
        st["h1"][:, t + 1 : t + 2], st["z1"][:, 1:2], st["cr"], st["cr"],
        OP.mult, OP.add,
    )
    if (t + 1) % RENORM == 0:
        _emit_renorm(nc, st, t)


def _emit_renorm(nc, st, t):
    """Rescale cn by 2^-e2(n) exactly; fold ln of the scale into future Gx_i."""
    cn_u = st["cn"][:, 1:2].bitcast(U32)
    nc.vector.tensor_scalar(
        st["p2"].bitcast(U32), cn_u, 0x7F800000, None, OP.bitwise_and
    )
    nc.vector.tensor_scalar(
        st["e2"].bitcast(U32), cn_u, 23, 0x4B000000,
        OP.logical_shift_right, OP.bitwise_or,
    )
    # negdelta = -(e_biased - 127) * ln2 ; e2 holds 2^23 + e_biased as fp32
    nc.vector.tensor_scalar(
        st["nd"], st["e2"], -8388735.0, -LN2, OP.add, OP.mult
    )
    nc.vector.tensor_tensor(st["negmu"], st["negmu"], st["nd"], OP.add)
    nc.vector.reciprocal(st["rs"], st["p2"])
    nc.vector.tensor_scalar(st["cn"], st["cn"], st["rs"], None, OP.mult)
    if t + 1 < T:
        hi = min(t + 1 + RENORM, T)
        sl = st["Gx"][:, t + 1 : hi, 0:1]
        nc.vector.tensor_scalar(sl, sl, st["negmu"], None, OP.add)


def _mlstm_chunk_ops(nc, st, ci, psB, chk, hout_d):
    """Return a list of closures, each emitting one instruction of mLSTM
    chunk ci. Layouts: channel on partitions ([a, t]) except where noted."""
    s0 = ci * L
    sl = slice(s0, s0 + L)
    h1sl = slice(1 + s0, 1 + s0 + L)
    last = ci == NCHUNK - 1
    ops = []

    # -- projections q,k,v,it,ft,to (o pre-halved, k pre-scaled)
    PROJ = [("q_", AF.Identity), ("k_", AF.Identity), ("v_", AF.Identity),
            ("it_", AF.Identity), ("ft_", AF.Identity), ("tom", AF.Tanh)]

    def mk_proj(j, name, func):
        def mm():
            ps = psB.tile([H, L], FP32, tag="ps", name="proj_ps")
            chk["proj_ps"] = ps
            nc.tensor.matmul(
                ps, st["WT6"][:, j * H : (j + 1) * H], st["h1"][:, h1sl],
                start=True, stop=True,
            )
        def cp():
            nc.scalar.activation(
                st[name][:, sl], chk["proj_ps"], func,
                bias=st["b6"][:, j : j + 1],
            )
        return [mm, cp]

    for j, (name, func) in enumerate(PROJ):
        ops += mk_proj(j, name, func)

    # -- gate scans: F = cumsum(ft); a = it - F; u = runmax(0, a)
    def scan_F():
        init = 0.0 if ci == 0 else st["F_"][:, s0 - 1 : s0]
        nc.vector.tensor_tensor_scan(
            st["F_"][:, sl], st["ft_"][:, sl], st["zerL"], init, OP.add, OP.add
        )
    def calc_a():
        nc.vector.tensor_tensor(
            st["a_"][:, sl], st["it_"][:, sl], st["F_"][:, sl], OP.subtract
        )
    def scan_u():
        init = 0.0 if ci == 0 else st["u_"][:, s0 - 1 : s0]
        nc.vector.tensor_tensor_scan(
            st["u_"][:, sl], st["a_"][:, sl], st["zerL"], init, OP.max, OP.add
        )
    ops += [scan_F, calc_a, scan_u]

    u_end = st["u_"][:, s0 + L - 1 : s0 + L]

    def calc_negu():
        nc.vector.tensor_scalar(st["negu"], u_end, -1.0, None, OP.mult)
    def calc_P():
        nc.scalar.activation(st["Pc"], st["a_"][:, sl], AF.Exp, bias=st["negu"])
    def calc_E():
        nc.scalar.activation(st["Ec"], st["u_"][:, sl], AF.Exp,
                             bias=u_end, scale=-1.0)
    ops += [calc_negu, calc_P, calc_E]

    if ci > 0:
        def calc_d():
            nc.scalar.activation(st["ddec"], st["u_"][:, s0 - 1 : s0], AF.Exp,
                                 bias=st["negu"])
        def scale_Cs():
            nc.vector.tensor_scalar(st["CsS"], st["Cs"], st["ddec"], None, OP.mult)
        def tr_Cs():
            ps = psB.tile([H, H], FP32, tag="ps2", name="cst_ps")
            chk["cst_ps"] = ps
            nc.tensor.transpose(ps, st["CsS"], st["ident"])
        def cp_Cst():
            nc.vector.tensor_copy(st["Cst"], chk["cst_ps"])
        def calc_dn():
            nc.vector.tensor_scalar(
                st["dn"], st["Ncum"][:, s0 - 1 : s0], st["ddec"], None, OP.mult
            )
        ops += [calc_d, scale_Cs, tr_Cs, cp_Cst, calc_dn]

    # -- n accumulation (per-channel cumsum of P*k with decayed carry)
    def calc_PK():
        nc.vector.tensor_tensor(st["PKc"], st["Pc"], st["k_"][:, sl], OP.mult)
    def scan_N():
        init = 0.0 if ci == 0 else st["dn"]
        nc.vector.tensor_tensor_scan(
            st["Ncum"][:, sl], st["PKc"], st["zerL"], init, OP.add, OP.add
        )
    ops += [calc_PK, scan_N]

    # -- attention-style intra-chunk matmuls
    def mm_St():
        ps = psB.tile([L, L], FP32, tag="ps2", name="st_ps")
        chk["st_ps"] = ps
        nc.tensor.matmul(ps, st["k_"][:, sl], st["q_"][:, sl],
                         start=True, stop=True)
    def mask_S():
        nc.vector.tensor_tensor(st["Sm"], chk["st_ps"], st["tri"], OP.mult)
    def calc_PV():
        nc.vector.tensor_tensor(st["PVa"], st["Pc"], st["v_"][:, sl], OP.mult)
    def tr_PV():
        ps = psB.tile([H, L], FP32, tag="ps2", name="t_ps")
        chk["pvt_ps"] = ps
        nc.tensor.transpose(ps, st["PVa"], st["ident"])
    def cp_PVt():
        nc.vector.tensor_copy(st["PVt"], chk["pvt_ps"])
    ops += [mm_St, mask_S, calc_PV, tr_PV, cp_PVt]

    def mm_IH():
        ps = psB.tile([L, H], FP32, tag="ps3", name="ih_ps")
        chk["ih_ps"] = ps
        nc.tensor.matmul(ps, st["Sm"], st["PVt"], start=True, stop=(ci == 0))
    ops.append(mm_IH)
    if ci > 0:
        def mm_carry():
            nc.tensor.matmul(chk["ih_ps"], st["q_"][:, sl], st["Cst"],
                             start=False, stop=True)
        ops.append(mm_carry)

    def cp_IH():
        nc.vector.tensor_copy(st["IHs"], chk["ih_ps"])
    def tr_IH():
        ps = psB.tile([H, L], FP32, tag="ps2", name="t_ps")
        chk["iht_ps"] = ps
        nc.tensor.transpose(ps, st["IHs"], st["ident"])
    ops += [cp_IH, tr_IH]

    # -- denominator: row = sum_a E*Ncum*q ; rec = 0.5/max(|row|, 1)
    def calc_ENQ():
        nc.vector.tensor_tensor(st["ENQ"], st["Ncum"][:, sl], st["q_"][:, sl],
                                OP.mult)
    def calc_ENQ2():
        nc.vector.tensor_tensor(st["ENQ2"], st["ENQ"], st["Ec"], OP.mult)
    def mm_row():
        ps = psB.tile([1, L], FP32, tag="ps4", name="row_ps")
        chk["row_ps"] = ps
        nc.tensor.matmul(ps, st["ones1"], st["ENQ2"], start=True, stop=True)
    def calc_drow():
        nc.scalar.activation(st["drow"], chk["row_ps"], AF.Abs)
    def calc_drow2():
        nc.vector.tensor_scalar(st["drow2"], st["drow"], 1.0, 2.0,
                                OP.max, OP.mult)
    def calc_rrow():
        nc.vector.reciprocal(st["rrow"], st["drow2"])
    def bcast_r():
        nc.gpsimd.partition_broadcast(st["Rb"], st["rrow"])
    ops += [calc_ENQ, calc_ENQ2, mm_row, calc_drow, calc_drow2, calc_rrow,
            bcast_r]

    # -- output: h = (1+to) * E * IH * (0.5/den)
    def calc_EH():
        nc.vector.tensor_tensor(st["EH"], st["Ec"], chk["iht_ps"], OP.mult)
    def calc_EHR():
        nc.vector.tensor_tensor(st["EHR"], st["EH"], st["Rb"], OP.mult)
    def calc_t2():
        nc.vector.tensor_scalar(st["t2"], st["tom"][:, sl], 1.0, None, OP.add)
    def calc_h():
        nc.vector.tensor_tensor(st["houts"][:, sl], st["t2"], st["EHR"], OP.mult)
    def dma_h():
        nc.sync.dma_start(out=hout_d[:, sl], in_=st["houts"][:, sl])
    ops += [calc_EH, calc_EHR, calc_t2, calc_h, dma_h]

    # -- state update for next chunk
    if not last:
        def tr_K():
            ps = psB.tile([H, L], FP32, tag="ps2", name="t_ps")
            chk["kt_ps"] = ps
            nc.tensor.transpose(ps, st["k_"][:, sl], st["ident"])
        def cp_Kt():
            nc.vector.tensor_copy(st["Kts"], chk["kt_ps"])
        def mm_Cd():
            ps = psB.tile([H, H], FP32, tag="ps3", name="cd_ps")
            chk["cd_ps"] = ps
            nc.tensor.matmul(ps, st["PVt"], st["Kts"], start=True, stop=True)
        ops += [tr_K, cp_Kt, mm_Cd]
        if ci == 0:
            def upd_Cs():
                nc.vector.tensor_copy(st["Cs"], chk["cd_ps"])
        else:
            def upd_Cs():
                nc.vector.tensor_tensor(st["Cs"], st["CsS"], chk["cd_ps"], OP.add)
        ops.append(upd_Cs)

    return ops


def _build_body(nc, tc, dram):
    from contextlib import ExitStack

    with ExitStack() as ctx:
        const = ctx.enter_context(tc.tile_pool(name="const", bufs=1))
        psG = ctx.enter_context(tc.tile_pool(name="psG", bufs=2, space="PSUM"))
        psA = ctx.enter_context(tc.tile_pool(name="psA", bufs=2, space="PSUM"))
        psB = ctx.enter_context(tc.tile_pool(name="psB", bufs=1, space="PSUM"))

        st = {}

        def sb(name, shape, dtype=FP32):
            st[name] = const.tile(shape, dtype, tag=name, name=name)
            return st[name]

        # constants / weights
        for name, shape in [
            ("xT", [I, T]), ("sWT4", [I, 4 * H]), ("sRT4", [H, 4 * H]),
            ("sb4", [H, 4]), ("WT6", [H, 6 * H]), ("b6", [H, 6]),
        ]:
            sb(name, shape)
            nc.sync.dma_start(out=st[name], in_=dram[name][:])
        ident = sb("ident", [128, 128]); make_identity(nc, ident[:, :])
        tri = sb("tri", [L, L]); make_upper_triangular(nc, tri[:, :], val=1.0, diag=True)
        sb("zerL", [128, L]); nc.vector.memset(st["zerL"], 0.0)
        sb("ones1", [128, 1]); nc.vector.memset(st["ones1"], 1.0)

        # persistent buffers
        sb("Gx", [H, T, 4])
        sb("h1", [H, T + 1]); nc.vector.memset(st["h1"][:, 0:1], 0.0)
        for name in ["q_", "k_", "v_", "it_", "ft_", "tom", "F_", "a_", "u_",
                     "Ncum", "houts"]:
            sb(name, [H, T])
        for name in ["Cs", "CsS", "Cst"]:
            sb(name, [H, H])
        # sLSTM step state
        sb("cn", [H, 2]); nc.vector.memset(st["cn"], 0.0)
        sb("z1", [H, 3]); nc.vector.memset(st["z1"][:, 2:3], 1.0)
        for name in ["eif", "iz1"]:
            sb(name, [H, 2])
        for name in ["rr", "cr", "p2", "e2", "nd", "negmu", "rs"]:
            sb(name, [H, 1])
        nc.vector.memset(st["negmu"], 0.0)
        # mLSTM chunk scratch
        for name in ["Pc", "Ec", "PKc", "Sm", "PVa", "PVt", "IHs", "Kts",
                     "ENQ", "ENQ2", "Rb", "EH", "EHR", "t2"]:
            sb(name, [128, L])
        for name in ["negu", "ddec", "dn"]:
            sb(name, [H, 1])
        sb("drow", [1, L]); sb("drow2", [1, L]); sb("rrow", [1, L])

        # Gx precompute: Gx[:, tt, g] = sW_g @ x_t (+ sb_g)
        for g in range(4):
            for tt in range(T // 512):
                ps = psA.tile([H, 512], FP32, tag="gx", name="gx_ps")
                nc.tensor.matmul(
                    ps, st["sWT4"][:, g * H : (g + 1) * H],
                    st["xT"][:, tt * 512 : (tt + 1) * 512],
                    start=True, stop=True,
                )
                nc.scalar.activation(
                    st["Gx"][:, tt * 512 : (tt + 1) * 512, g], ps,
                    AF.Identity, bias=st["sb4"][:, g : g + 1],
                )

        # serial loop with interleaved mLSTM chunk work
        chk = {}
        pending = []
        for t in range(T):
            _emit_slstm_step(nc, st, t, psG)
            if pending:
                pending.pop(0)()
            if (t + 1) % L == 0:
                ci = (t + 1) // L - 1
                pending += _mlstm_chunk_ops(nc, st, ci, psB, chk, dram["hout"])
        while pending:
            pending.pop(0)()


def _get_nc():
    if "nc" in _NC_CACHE:
        return _NC_CACHE["nc"]
    nc = bacc.Bacc("TRN2", debug=False, num_devices=B)
    dram = {}
    for name, shape in [
        ("xT", [I, T]), ("sWT4", [I, 4 * H]), ("sRT4", [H, 4 * H]),
        ("sb4", [H, 4]), ("WT6", [H, 6 * H]), ("b6", [H, 6]),
    ]:
        dram[name] = nc.declare_dram_parameter(name, shape, FP32, isOutput=False)
    dram["hout"] = nc.declare_dram_parameter("hout", [H, T], FP32, isOutput=True)
    with tile.TileContext(nc) as tc:
        _build_body(nc, tc, dram)
    nc.compile()
    _NC_CACHE["nc"] = nc
    return nc


def _make_runner(nc):
    """Build a jitted SPMD runner for a compiled Bacc program (replicates
    bass2jax.run_bass_via_pjrt but reuses the jitted callable across calls)."""
    import jax
    from jax.sharding import Mesh, PartitionSpec
    from jax.experimental.shard_map import shard_map
    from concourse import mybir as _mb
    from concourse.bass2jax import (
        _bass_exec_p, install_neuronx_cc_hook, partition_id_tensor,
    )

    install_neuronx_cc_hook()
    partition_name = nc.partition_id_tensor.name if nc.partition_id_tensor else None
    in_names, out_names, out_avals, zero_outs = [], [], [], []
    for alloc in nc.m.functions[0].allocations:
        if not isinstance(alloc, _mb.MemoryLocationSet):
            continue
        name = alloc.memorylocations[0].name
        if alloc.kind == "ExternalInput":
            if name != partition_name:
                in_names.append(name)
        elif alloc.kind == "ExternalOutput":
            out_names.append(name)
            shape = tuple(alloc.tensor_shape)
            dtype = _mb.dt.np(alloc.dtype)
            out_avals.append(jax.core.ShapedArray(shape, dtype))
            zero_outs.append(np.zeros(shape, dtype))
    n_params = len(in_names)
    n_outs = len(out_avals)
    param_names = list(in_names)
    in_names = in_names + out_names
    if partition_name is not None:
        in_names.append(partition_name)

    def _body(*args):
        operands = list(args)
        if partition_name is not None:
            operands.append(partition_id_tensor())
        outs = _bass_exec_p.bind(
            *operands,
            out_avals=tuple(out_avals),
            in_names=tuple(in_names),
            out_names=tuple(out_names),
            lowering_input_output_aliases=(),
            sim_require_finite=True,
            sim_require_nnan=True,
            nc=nc,
        )
        return tuple(outs)

    devices = jax.devices()[:B]
    mesh = Mesh(np.asarray(devices), ("core",))
    in_specs = (PartitionSpec("core"),) * (n_params + n_outs)
    out_specs = (PartitionSpec("core"),) * n_outs
    sharded = jax.jit(
        shard_map(_body, mesh=mesh, in_specs=in_specs, out_specs=out_specs,
                  check_rep=False),
        donate_argnums=tuple(range(n_params, n_params + n_outs)),
        keep_unused=True,
    )

    def run(in_maps):
        concat_in = [
            np.concatenate([np.asarray(m[name]) for m in in_maps], axis=0)
            for name in param_names
        ]
        concat_zeros = [
            np.zeros((B * z.shape[0], *z.shape[1:]), z.dtype) for z in zero_outs
        ]
        out_arrs = sharded(*concat_in, *concat_zeros)
        out_arrs = [np.asarray(a) for a in out_arrs]
        return [
            {name: out_arrs[i].reshape(B, *out_avals[i].shape)[c]
             for i, name in enumerate(out_names)}
            for c in range(B)
        ]

    return run


def _get_runner():
    if "runner" not in _NC_CACHE:
        _NC_CACHE["runner"] = _make_runner(_get_nc())
    return _NC_CACHE["runner"]


def kernel(**inputs):
    global LAST_RESULTS
    f32 = np.float32
    x = np.ascontiguousarray(inputs["x"], dtype=f32)
    sW = np.asarray(inputs["sW"], f32); sR = np.asarray(inputs["sR"], f32)
    sb_ = np.asarray(inputs["sb"], f32)
    inv_sqrt_h = f32(1.0 / np.sqrt(H))

    sWT4 = np.ascontiguousarray(sW.T); sRT4 = np.ascontiguousarray(sR.T)
    sWT4[:, 3 * H :] *= 0.5; sRT4[:, 3 * H :] *= 0.5
    sb4 = np.ascontiguousarray(sb_.reshape(4, H).T)
    sb4[:, 3] *= 0.5

    WT = {}
    bvecs = []
    for j, wn, bn in [(0, "Wq", "bq"), (1, "Wk", "bk"), (2, "Wv", "bv"),
                      (3, "Wi", "bi"), (4, "Wf", "bf"), (5, "Wo", "bo")]:
        w = np.asarray(inputs[wn], f32).T.copy()
        b = np.asarray(inputs[bn], f32).copy()
        if wn == "Wk":
            w *= inv_sqrt_h; b = b * inv_sqrt_h
        if wn == "Wo":
            w *= 0.5; b = b * 0.5
        WT[j] = w
        bvecs.append(b)
    WT6 = np.ascontiguousarray(np.concatenate([WT[j] for j in range(6)], axis=1))
    b6 = np.ascontiguousarray(np.stack(bvecs, axis=1))

    global LAST_IN_MAPS
    run = _get_runner()
    in_maps = []
    for b_ in range(B):
        in_maps.append({
            "xT": np.ascontiguousarray(x[b_].T),
            "sWT4": sWT4, "sRT4": sRT4, "sb4": sb4, "WT6": WT6, "b6": b6,
        })
    LAST_IN_MAPS = in_maps
    results = run(in_maps)
    LAST_RESULTS = results
    out = np.empty((B, T, H), f32)
    for b_ in range(B):
        out[b_] = results[b_]["hout"].T
    return out



# revision 13
# speedup vs baseline: 35.6749x; 1.3499x over previous
"""Trainium2 Bass kernel for a 2-layer xLSTM (sLSTM -> mLSTM).

Strategy (8 NeuronCores, data-parallel over batch, 1 batch element/core):
  - Layer 0 (sLSTM): serial scan over T=1024. State kept UNNORMALIZED
    (c,n scaled by exp(m)); every RENORM steps the state is rescaled by
    1/n (reusing the per-step reciprocal) and ln(n) is folded into the
    future i-gate pre-activations (Gx patch).
    Per step: Gx injected into PSUM by the scalar engine, 4 bf16 gate
    matvecs accumulate on top (order z,i,f,o so activations overlap the
    PE stream), then Tanh(z)/Exp(i,f)/Sigmoid(o) + a 4-op vector tail.
  - Layer 1 (mLSTM): chunkwise-parallel formulation (chunk L=128) with
    bf16 matmuls. Work for chunk c is interleaved into the serial sLSTM
    steps of chunk c+1 where the engines are otherwise idle.

kernel(**inputs) takes the FULL inputs and returns the FULL (B,T,H) output.
"""

import numpy as np
import ml_dtypes

import concourse.bacc as bacc
import concourse.tile as tile
from concourse import mybir
from concourse.masks import make_identity, make_upper_triangular

AF = mybir.ActivationFunctionType
OP = mybir.AluOpType
FP32 = mybir.dt.float32
BF16 = mybir.dt.bfloat16
U32 = mybir.dt.uint32

B, T, I, H = 8, 1024, 128, 128
L = 128
NCHUNK = T // L
RENORM = 32
LN2 = 0.6931471805599453

TRACE = False
LAST_RESULTS = None
LAST_IN_MAPS = None
_NC_CACHE = {}


def _emit_gx_inject(nc, st, t, psG):
    """Open PSUM group for step t and inject Gx[:, t, :] (off critical path)."""
    gps = psG.tile([H, 4], FP32, tag="g", name="gps")
    st["gps_cur"] = gps
    nc.tensor.matmul(gps, st["ident"], st["Gx"][:, t, :], start=True, stop=False)


def _emit_slstm_step(nc, st, t, psG):
    """One serial sLSTM step. h1[:, t+1] <- step(h1[:, t])."""
    gps = st["gps_cur"]
    # split-precision gate matvecs: R16*h16 + R16*dh + dR16*h16, order z,o,i,f
    for g in (2, 3, 0, 1):
        col = gps[:, g : g + 1]
        r16 = st["sRT4b"][:, g * H : (g + 1) * H]
        nc.tensor.matmul(col, r16, st["h1"][:, t : t + 1],
                         start=False, stop=False, skip_group_check=True)
        nc.tensor.matmul(col, r16, st["dh1"][:, t : t + 1],
                         start=False, stop=False, skip_group_check=True)
        nc.tensor.matmul(col, st["sRD"][:, g * H : (g + 1) * H],
                         st["h1"][:, t : t + 1],
                         start=False, stop=(g == 1), skip_group_check=True)
    # eif = exp(psum[:, i,f]); z1[:,0:2] = tanh(psum[:, z,o])
    nc.scalar.activation(st["eif"], gps[:, 0:2], AF.Exp)
    nc.scalar.activation(st["z1"][:, 0:2], gps[:, 2:4], AF.Tanh)
    # iz1 = ei * [tz, 1]
    nc.vector.tensor_scalar(
        st["iz1"], st["z1"][:, 0:3:2], st["eif"][:, 0:1], None, OP.mult
    )
    # cn = ef*cn + iz1
    nc.vector.scalar_tensor_tensor(
        st["cn"], st["cn"], st["eif"][:, 1:2], st["iz1"], OP.mult, OP.add
    )
    nc.vector.reciprocal(st["rr"], st["cn"][:, 1:2])
    # cr = 0.5 * c / n
    nc.vector.tensor_scalar(
        st["cr"], st["cn"][:, 0:1], st["rr"], 0.5, OP.mult, OP.mult
    )
    # h = to*cr + cr  (= sigmoid(o)*c/n with o pre-halved)
    nc.vector.scalar_tensor_tensor(
        st["hf"], st["z1"][:, 1:2], st["cr"], st["cr"], OP.mult, OP.add,
    )
    nc.vector.tensor_copy(st["h1"][:, t + 1 : t + 2], st["hf"])
    nc.vector.tensor_tensor(
        st["dh1"][:, t + 1 : t + 2], st["hf"], st["h1"][:, t + 1 : t + 2],
        OP.subtract,
    )
    if (t + 1) % RENORM == 0:
        _emit_renorm(nc, st, t)
    if t + 1 < T:
        _emit_gx_inject(nc, st, t + 1, psG)


def _emit_renorm(nc, st, t):
    """Rescale cn by 2^-e2(n) exactly; fold ln of the scale into future Gx_i."""
    cn_u = st["cn"][:, 1:2].bitcast(U32)
    nc.vector.tensor_scalar(
        st["p2"].bitcast(U32), cn_u, 0x7F800000, None, OP.bitwise_and
    )
    nc.vector.tensor_scalar(
        st["e2"].bitcast(U32), cn_u, 23, 0x4B000000,
        OP.logical_shift_right, OP.bitwise_or,
    )
    # negdelta = -(e_biased - 127) * ln2 ; e2 holds 2^23 + e_biased as fp32
    nc.vector.tensor_scalar(
        st["nd"], st["e2"], -8388735.0, -LN2, OP.add, OP.mult
    )
    nc.vector.tensor_tensor(st["negmu"], st["negmu"], st["nd"], OP.add)
    nc.vector.reciprocal(st["rs"], st["p2"])
    nc.vector.tensor_scalar(st["cn"], st["cn"], st["rs"], None, OP.mult)
    if t + 1 < T:
        hi = min(t + 1 + RENORM, T)
        sl = st["Gx"][:, t + 1 : hi, 0:1]
        nc.vector.tensor_scalar(sl, sl, st["negmu"], None, OP.add)


def _mlstm_chunk_ops(nc, st, ci, psB, chk, hout_d):
    """Return a list of closures, each emitting one instruction of mLSTM
    chunk ci. Layouts: channel on partitions ([a, t]) except where noted."""
    s0 = ci * L
    sl = slice(s0, s0 + L)
    h1sl = slice(1 + s0, 1 + s0 + L)
    last = ci == NCHUNK - 1
    ops = []

    # -- projections q,k,v,it,ft,som (k pre-scaled by 1/sqrt(H))
    PROJ = [("q_", AF.Identity), ("k_", AF.Identity), ("v_", AF.Identity),
            ("it_", AF.Identity), ("ft_", AF.Identity), ("tom", AF.Tanh)]

    def mk_proj(j, name, func):
        def mm():
            ps = psB.tile([H, L], FP32, tag="ps", name="proj_ps")
            chk["proj_ps"] = ps
            nc.tensor.matmul(
                ps, st["WT6b"][:, j * H : (j + 1) * H], st["h1"][:, h1sl],
                start=True, stop=True,
            )
        def cp():
            nc.scalar.activation(
                st[name][:, sl], chk["proj_ps"], func,
                bias=st["b6"][:, j : j + 1],
            )
        return [mm, cp]

    for j, (name, func) in enumerate(PROJ):
        ops += mk_proj(j, name, func)

    # -- gate scans: F = cumsum(ft); a = it - F; u = runmax(0, a)
    def scan_F():
        init = 0.0 if ci == 0 else st["F_"][:, s0 - 1 : s0]
        nc.vector.tensor_tensor_scan(
            st["F_"][:, sl], st["ft_"][:, sl], st["zerL"], init, OP.add, OP.add
        )
    def calc_a():
        nc.vector.tensor_tensor(
            st["a_"][:, sl], st["it_"][:, sl], st["F_"][:, sl], OP.subtract
        )
    def scan_u():
        init = 0.0 if ci == 0 else st["u_"][:, s0 - 1 : s0]
        nc.vector.tensor_tensor_scan(
            st["u_"][:, sl], st["a_"][:, sl], st["zerL"], init, OP.max, OP.add
        )
    ops += [scan_F, calc_a, scan_u]

    u_end = st["u_"][:, s0 + L - 1 : s0 + L]

    def calc_negu():
        nc.vector.tensor_scalar(st["negu"], u_end, -1.0, None, OP.mult)
    def calc_P():
        nc.scalar.activation(st["Pc"], st["a_"][:, sl], AF.Exp, bias=st["negu"])
    def calc_E():
        nc.scalar.activation(st["Ec"], st["u_"][:, sl], AF.Exp,
                             bias=u_end, scale=-1.0)
    ops += [calc_negu, calc_P, calc_E]

    if ci > 0:
        def calc_d():
            nc.scalar.activation(st["ddec"], st["u_"][:, s0 - 1 : s0], AF.Exp,
                                 bias=st["negu"])
        def scale_Cs():
            nc.vector.tensor_scalar(st["CsS"], st["Cs"], st["ddec"], None, OP.mult)
        def tr_Cs():
            ps = psB.tile([H, H], FP32, tag="ps2", name="cst_ps")
            chk["cst_ps"] = ps
            nc.tensor.transpose(ps, st["CsS"], st["ident"])
        def cp_Cst():
            nc.vector.tensor_copy(st["Cst"], chk["cst_ps"])
        def calc_dn():
            nc.vector.tensor_scalar(
                st["dn"], st["Ncum"][:, s0 - 1 : s0], st["ddec"], None, OP.mult
            )
        ops += [calc_d, scale_Cs, tr_Cs, cp_Cst, calc_dn]

    # -- n accumulation (per-channel cumsum of P*k with decayed carry)
    def calc_PK():
        nc.vector.tensor_tensor(st["PKc"], st["Pc"], st["k_"][:, sl], OP.mult)
    def scan_N():
        init = 0.0 if ci == 0 else st["dn"]
        nc.vector.tensor_tensor_scan(
            st["Ncum"][:, sl], st["PKc"], st["zerL"], init, OP.add, OP.add
        )
    ops += [calc_PK, scan_N]

    # -- attention-style intra-chunk matmuls
    def mm_St():
        ps = psB.tile([L, L], FP32, tag="ps2", name="st_ps")
        chk["st_ps"] = ps
        nc.tensor.matmul(ps, st["k_"][:, sl], st["q_"][:, sl],
                         start=True, stop=True)
    def mask_S():
        nc.vector.tensor_tensor(st["Sm"], chk["st_ps"], st["tri"], OP.mult)
    def calc_PV():
        nc.vector.tensor_tensor(st["PVa"], st["Pc"], st["v_"][:, sl], OP.mult)
    def tr_PV():
        ps = psB.tile([H, L], BF16, tag="ps2", name="t_ps")
        chk["pvt_ps"] = ps
        nc.tensor.transpose(ps, st["PVa"], st["identb"])
    def cp_PVt():
        nc.vector.tensor_copy(st["PVt"], chk["pvt_ps"])
    ops += [mm_St, mask_S, calc_PV, tr_PV, cp_PVt]

    def mm_IH():
        ps = psB.tile([L, H], FP32, tag="ps3", name="ih_ps")
        chk["ih_ps"] = ps
        nc.tensor.matmul(ps, st["Sm"], st["PVt"], start=True, stop=(ci == 0))
    ops.append(mm_IH)
    if ci > 0:
        def mm_carry():
            nc.tensor.matmul(chk["ih_ps"], st["q_"][:, sl], st["Cst"],
                             start=False, stop=True)
        ops.append(mm_carry)

    def cp_IH():
        nc.vector.tensor_copy(st["IHs"], chk["ih_ps"])
    def tr_IH():
        ps = psB.tile([H, L], BF16, tag="ps2", name="t_ps")
        chk["iht_ps"] = ps
        nc.tensor.transpose(ps, st["IHs"], st["identb"])
    ops += [cp_IH, tr_IH]

    # -- denominator: row = sum_a E*Ncum*q ; rec = 1/max(|row|, 1)
    def calc_ENQ():
        nc.vector.tensor_tensor(st["ENQ"], st["Ncum"][:, sl], st["q_"][:, sl],
                                OP.mult)
    def calc_ENQ2():
        nc.vector.tensor_tensor(st["ENQ2"], st["ENQ"], st["Ec"], OP.mult)
    def mm_row():
        ps = psB.tile([1, L], FP32, tag="ps4", name="row_ps")
        chk["row_ps"] = ps
        nc.tensor.matmul(ps, st["ones1b"], st["ENQ2"], start=True, stop=True)
    def calc_drow():
        nc.scalar.activation(st["drow"], chk["row_ps"], AF.Abs)
    def calc_drow2():
        nc.vector.tensor_scalar(st["drow2"], st["drow"], 1.0, 2.0,
                                OP.max, OP.mult)
    def calc_rrow():
        nc.vector.reciprocal(st["rrow"], st["drow2"])
    def bcast_r():
        nc.gpsimd.partition_broadcast(st["Rb"], st["rrow"])
    ops += [calc_ENQ, calc_ENQ2, mm_row, calc_drow, calc_drow2, calc_rrow,
            bcast_r]

    # -- output: h = (1+to) * E * IH * (0.5/den)
    def calc_EH():
        nc.vector.tensor_tensor(st["EH"], st["Ec"], chk["iht_ps"], OP.mult)
    def calc_EHR():
        nc.vector.tensor_tensor(st["EHR"], st["EH"], st["Rb"], OP.mult)
    def calc_t2():
        nc.vector.tensor_scalar(st["t2"], st["tom"][:, sl], 1.0, None, OP.add)
    def calc_h():
        nc.vector.tensor_tensor(st["houts"][:, sl], st["t2"], st["EHR"], OP.mult)
    def dma_h():
        nc.sync.dma_start(out=hout_d[:, sl], in_=st["houts"][:, sl])
    ops += [calc_EH, calc_EHR, calc_t2, calc_h, dma_h]

    # -- state update for next chunk
    if not last:
        def tr_K():
            ps = psB.tile([H, L], BF16, tag="ps2", name="t_ps")
            chk["kt_ps"] = ps
            nc.tensor.transpose(ps, st["k_"][:, sl], st["identb"])
        def cp_Kt():
            nc.vector.tensor_copy(st["Kts"], chk["kt_ps"])
        def mm_Cd():
            ps = psB.tile([H, H], FP32, tag="ps3", name="cd_ps")
            chk["cd_ps"] = ps
            nc.tensor.matmul(ps, st["PVt"], st["Kts"], start=True, stop=True)
        ops += [tr_K, cp_Kt, mm_Cd]
        if ci == 0:
            def upd_Cs():
                nc.vector.tensor_copy(st["Cs"], chk["cd_ps"])
        else:
            def upd_Cs():
                nc.vector.tensor_tensor(st["Cs"], st["CsS"], chk["cd_ps"], OP.add)
        ops.append(upd_Cs)

    return ops


def _build_body(nc, tc, dram):
    from contextlib import ExitStack

    with ExitStack() as ctx:
        const = ctx.enter_context(tc.tile_pool(name="const", bufs=1))
        psG = ctx.enter_context(tc.tile_pool(name="psG", bufs=2, space="PSUM"))
        psA = ctx.enter_context(tc.tile_pool(name="psA", bufs=2, space="PSUM"))
        psB = ctx.enter_context(tc.tile_pool(name="psB", bufs=1, space="PSUM"))

        st = {}

        def sb(name, shape, dtype=FP32):
            st[name] = const.tile(shape, dtype, tag=name, name=name)
            return st[name]

        # constants / weights
        for name, shape, dt in [
            ("xT", [I, T], FP32), ("sWT4", [I, 4 * H], FP32),
            ("sRT4b", [H, 4 * H], BF16), ("sRD", [H, 4 * H], BF16),
            ("sb4", [H, 4], FP32),
            ("WT6b", [H, 6 * H], BF16), ("b6", [H, 6], FP32),
        ]:
            sb(name, shape, dt)
            nc.sync.dma_start(out=st[name], in_=dram[name][:])
        ident = sb("ident", [128, 128]); make_identity(nc, ident[:, :])
        identb = sb("identb", [128, 128], BF16); make_identity(nc, identb[:, :])
        tri = sb("tri", [L, L]); make_upper_triangular(nc, tri[:, :], val=1.0, diag=True)
        sb("zerL", [128, L]); nc.vector.memset(st["zerL"], 0.0)
        sb("ones1b", [128, 1], BF16); nc.vector.memset(st["ones1b"], 1.0)

        # persistent buffers
        sb("Gx", [H, T, 4])
        sb("h1", [H, T + 1], BF16); nc.vector.memset(st["h1"][:, 0:1], 0.0)
        sb("dh1", [H, T + 1], BF16); nc.vector.memset(st["dh1"][:, 0:1], 0.0)
        sb("hf", [H, 1])
        for name in ["it_", "ft_", "tom", "F_", "a_", "u_", "Ncum", "houts"]:
            sb(name, [H, T])
        for name in ["q_", "k_", "v_"]:
            sb(name, [H, T], BF16)
        sb("Cs", [H, H]); sb("CsS", [H, H]); sb("Cst", [H, H], BF16)
        # sLSTM step state
        sb("cn", [H, 2]); nc.vector.memset(st["cn"], 0.0)
        sb("z1", [H, 3]); nc.vector.memset(st["z1"][:, 2:3], 1.0)
        sb("eif", [H, 2]); sb("iz1", [H, 2])
        for name in ["rr", "cr", "p2", "e2", "nd", "negmu", "rs"]:
            sb(name, [H, 1])
        nc.vector.memset(st["negmu"], 0.0)
        # mLSTM chunk scratch
        for name in ["Pc", "Ec", "PKc", "ENQ", "Rb", "EH", "EHR", "t2"]:
            sb(name, [128, L])
        for name in ["Sm", "PVa", "PVt", "IHs", "Kts", "ENQ2"]:
            sb(name, [128, L], BF16)
        for name in ["negu", "ddec", "dn"]:
            sb(name, [H, 1])
        sb("drow", [1, L]); sb("drow2", [1, L]); sb("rrow", [1, L])

        # Gx precompute: Gx[:, tt, g] = sW_g @ x_t (+ sb_g)
        for g in range(4):
            for tt in range(T // 512):
                ps = psA.tile([H, 512], FP32, tag="gx", name="gx_ps")
                nc.tensor.matmul(
                    ps, st["sWT4"][:, g * H : (g + 1) * H],
                    st["xT"][:, tt * 512 : (tt + 1) * 512],
                    start=True, stop=True,
                )
                nc.scalar.activation(
                    st["Gx"][:, tt * 512 : (tt + 1) * 512, g], ps,
                    AF.Identity, bias=st["sb4"][:, g : g + 1],
                )

        # serial loop with interleaved mLSTM chunk work
        chk = {}
        pending = []
        _emit_gx_inject(nc, st, 0, psG)
        for t in range(T):
            _emit_slstm_step(nc, st, t, psG)
            if pending and t % 2 == 0:
                pending.pop(0)()
            if (t + 1) % L == 0:
                ci = (t + 1) // L - 1
                pending += _mlstm_chunk_ops(nc, st, ci, psB, chk, dram["hout"])
        while pending:
            pending.pop(0)()


def _get_nc():
    if "nc" in _NC_CACHE:
        return _NC_CACHE["nc"]
    nc = bacc.Bacc("TRN2", debug=False, num_devices=B)
    dram = {}
    for name, shape, dt in [
        ("xT", [I, T], FP32), ("sWT4", [I, 4 * H], FP32),
        ("sRT4b", [H, 4 * H], BF16), ("sRD", [H, 4 * H], BF16),
        ("sb4", [H, 4], FP32),
        ("WT6b", [H, 6 * H], BF16), ("b6", [H, 6], FP32),
    ]:
        dram[name] = nc.declare_dram_parameter(name, shape, dt, isOutput=False)
    dram["hout"] = nc.declare_dram_parameter("hout", [H, T], FP32, isOutput=True)
    with tile.TileContext(nc) as tc:
        _build_body(nc, tc, dram)
    nc.compile()
    _NC_CACHE["nc"] = nc
    return nc


def _make_runner(nc):
    """Build a jitted SPMD runner for a compiled Bacc program (replicates
    bass2jax.run_bass_via_pjrt but reuses the jitted callable across calls)."""
    import jax
    from jax.sharding import Mesh, PartitionSpec
    from jax.experimental.shard_map import shard_map
    from concourse import mybir as _mb
    from concourse.bass2jax import (
        _bass_exec_p, install_neuronx_cc_hook, partition_id_tensor,
    )

    install_neuronx_cc_hook()
    partition_name = nc.partition_id_tensor.name if nc.partition_id_tensor else None
    in_names, out_names, out_avals, zero_outs = [], [], [], []
    for alloc in nc.m.functions[0].allocations:
        if not isinstance(alloc, _mb.MemoryLocationSet):
            continue
        name = alloc.memorylocations[0].name
        if alloc.kind == "ExternalInput":
            if name != partition_name:
                in_names.append(name)
        elif alloc.kind == "ExternalOutput":
            out_names.append(name)
            shape = tuple(alloc.tensor_shape)
            dtype = _mb.dt.np(alloc.dtype)
            out_avals.append(jax.core.ShapedArray(shape, dtype))
            zero_outs.append(np.zeros(shape, dtype))
    n_params = len(in_names)
    n_outs = len(out_avals)
    param_names = list(in_names)
    in_names = in_names + out_names
    if partition_name is not None:
        in_names.append(partition_name)

    def _body(*args):
        operands = list(args)
        if partition_name is not None:
            operands.append(partition_id_tensor())
        outs = _bass_exec_p.bind(
            *operands,
            out_avals=tuple(out_avals),
            in_names=tuple(in_names),
            out_names=tuple(out_names),
            lowering_input_output_aliases=(),
            sim_require_finite=True,
            sim_require_nnan=True,
            nc=nc,
        )
        return tuple(outs)

    devices = jax.devices()[:B]
    mesh = Mesh(np.asarray(devices), ("core",))
    in_specs = (PartitionSpec("core"),) * (n_params + n_outs)
    out_specs = (PartitionSpec("core"),) * n_outs
    sharded = jax.jit(
        shard_map(_body, mesh=mesh, in_specs=in_specs, out_specs=out_specs,
                  check_rep=False),
        donate_argnums=tuple(range(n_params, n_params + n_outs)),
        keep_unused=True,
    )

    def run(in_maps):
        concat_in = [
            np.concatenate([np.asarray(m[name]) for m in in_maps], axis=0)
            for name in param_names
        ]
        concat_zeros = [
            np.zeros((B * z.shape[0], *z.shape[1:]), z.dtype) for z in zero_outs
        ]
        out_arrs = sharded(*concat_in, *concat_zeros)
        out_arrs = [np.asarray(a) for a in out_arrs]
        return [
            {name: out_arrs[i].reshape(B, *out_avals[i].shape)[c]
             for i, name in enumerate(out_names)}
            for c in range(B)
        ]

    return run


def _get_runner():
    if "runner" not in _NC_CACHE:
        _NC_CACHE["runner"] = _make_runner(_get_nc())
    return _NC_CACHE["runner"]


def kernel(**inputs):
    global LAST_RESULTS, LAST_IN_MAPS
    f32 = np.float32
    f16 = ml_dtypes.bfloat16
    x = np.ascontiguousarray(inputs["x"], dtype=f32)
    sW = np.asarray(inputs["sW"], f32); sR = np.asarray(inputs["sR"], f32)
    sb_ = np.asarray(inputs["sb"], f32)
    inv_sqrt_h = f32(1.0 / np.sqrt(H))

    sWT4 = np.ascontiguousarray(sW.T)
    sWT4[:, 3 * H :] *= 0.5
    sRT4 = np.ascontiguousarray(sR.T)
    sRT4[:, 3 * H :] *= 0.5
    sRT4b = sRT4.astype(f16)
    sRD = (sRT4 - sRT4b.astype(np.float32)).astype(f16)
    sb4 = np.ascontiguousarray(sb_.reshape(4, H).T)
    sb4[:, 3] *= 0.5

    WT = {}
    bvecs = []
    for j, wn, bn in [(0, "Wq", "bq"), (1, "Wk", "bk"), (2, "Wv", "bv"),
                      (3, "Wi", "bi"), (4, "Wf", "bf"), (5, "Wo", "bo")]:
        w = np.asarray(inputs[wn], f32).T.copy()
        b = np.asarray(inputs[bn], f32).copy()
        if wn == "Wk":
            w *= inv_sqrt_h; b = b * inv_sqrt_h
        if wn == "Wo":
            w = w * 0.5; b = b * 0.5
        WT[j] = w
        bvecs.append(b)
    WT6b = np.ascontiguousarray(
        np.concatenate([WT[j] for j in range(6)], axis=1)).astype(f16)
    b6 = np.ascontiguousarray(np.stack(bvecs, axis=1))

    run = _get_runner()
    in_maps = []
    for b_ in range(B):
        in_maps.append({
            "xT": np.ascontiguousarray(x[b_].T),
            "sWT4": sWT4, "sRT4b": sRT4b, "sRD": sRD, "sb4": sb4,
            "WT6b": WT6b, "b6": b6,
        })
    LAST_IN_MAPS = in_maps
    results = run(in_maps)
    LAST_RESULTS = results
    out = np.empty((B, T, H), f32)
    for b_ in range(B):
        out[b_] = results[b_]["hout"].T
    return out
